# revision 17
# baseline (speedup 1.0000x reference)
"""Trainium2 Bass kernel: dense transformer attention block (QKV proj + RoPE +
GQA causal attention + output proj), tensor-parallel over 8 NeuronCores.

Sharding: heads are split across cores (4 Q heads + 1 KV head per core).
Each core computes its QKV shard for all tokens and runs attention for its
heads. The per-core attention outputs (head-sharded) are then re-sharded to
token-sharded via one AllToAll per batch (each core keeps a 256-token slice
of every batch), after which each core computes the FULL output projection
(all 4096 columns) for its own tokens, streaming w_o from HBM. The host
concatenates the per-core token slices — no AllGather / AllReduce needed,
cutting collective traffic 8x vs gathering the full attention output.

All token-indexed tensors live on-device in transposed layout
([feature, token]) so the hidden-dim contraction lands on the partition axis
for the TensorEngine; the host transposes inputs/outputs during
shard/unshard.
"""

from contextlib import ExitStack

import numpy as np
import ml_dtypes

import concourse.bass as bass
from concourse import bacc
import concourse.tile as tile
import concourse.mybir as mybir
from concourse.bass_utils import run_bass_kernel_spmd

F32 = mybir.dt.float32
F32R = mybir.dt.float32r
BF16 = mybir.dt.bfloat16
EXP = mybir.ActivationFunctionType.Exp

N_CORES = 8
N_HEADS = 32
N_KV_HEADS = 8
D = 128          # head dim
HID = 4096
B = 2
S = 2048
T = B * S        # 4096 tokens
ROPE_BASE = 10000.0

HL = N_HEADS // N_CORES          # 4 local Q heads per core
QKV_ROWS = (HL + 2) * D          # 768: 4 Q heads + 1 K head + 1 V head
OWN = S // N_CORES               # 256 tokens owned per core per batch

TC = 512                         # token chunk for the QKV projection phase
QC = 512                         # query chunk in attention
N_HT = HID // 128                # 32 hidden tiles
N_JT = HID // 128                # 32 output column tiles in o_proj


def _emit(tc_ctx, xt, wqkvt, wot, ropes, out_t, qkt, a2a_ins, a2a_outs):
    nc = tc_ctx.nc
    n_tc = T // TC           # 8 projection chunks
    n_kt = S // 128          # 16 k-tiles per batch
    n_qc = S // QC           # 4 q-chunks per batch

    with ExitStack() as es:
        const_pool = es.enter_context(tc_ctx.tile_pool(name="const", bufs=1))
        # All-ones stationary: one matmul sums colsum partials over partitions
        # AND broadcasts the result across all 128 partitions.
        ones_mat = const_pool.tile([128, 128], BF16)
        # Diagonal causal masks: mask_d[k, q] = 1.0 if q - k - 128*d >= 0.
        masks = const_pool.tile([128, 4, QC], BF16)
        nc.vector.memset(masks, 1.0)
        nc.vector.tensor_copy(ones_mat, masks[:, 0, 0:128])
        for d_off in range(4):
            nc.gpsimd.affine_select(
                out=masks[:, d_off, :],
                in_=masks[:, d_off, :],
                compare_op=mybir.AluOpType.is_ge,
                fill=0.0,
                base=-128 * d_off,
                pattern=[[1, QC]],
                channel_multiplier=-1,
            )

        # Pools for attention inputs, opened early so batch-0 K/V/Q loads can
        # overlap the tail of phase 1.
        qpool = es.enter_context(tc_ctx.tile_pool(name="p2_q", bufs=2))
        kvpool = es.enter_context(tc_ctx.tile_pool(name="p2_kv", bufs=2))
        kvq = {}

        def load_kvq(b):
            k_sb = kvpool.tile([128, S], BF16, tag="k", name=f"k_sb{b}")
            nc.sync.dma_start(
                out=k_sb, in_=qkt[HL * 128:(HL + 1) * 128, b * S:(b + 1) * S]
            )
            v_sb = kvpool.tile([128, n_kt, 128], BF16, tag="v", name=f"v_sb{b}")
            for kt in range(n_kt):
                nc.scalar.dma_start(
                    out=v_sb[:, kt, :],
                    in_=qkt[(HL + 1) * 128:(HL + 2) * 128,
                            b * S + kt * 128:b * S + (kt + 1) * 128],
                    transpose=True,
                )
            q_sb = qpool.tile([128, HL, S], BF16, tag="q", name=f"q_sb{b}")
            nc.sync.dma_start(
                out=q_sb,
                in_=qkt.rearrange("(hh p) t -> p hh t", p=128)[
                    :, 0:HL, b * S:(b + 1) * S
                ],
            )
            kvq[b] = (k_sb, v_sb, q_sb)

        # ---------------- Phase 1: QKV projection + RoPE --------------------
        with tc_ctx.tile_pool(name="p1_w", bufs=1) as wpool, \
             tc_ctx.tile_pool(name="p1_x", bufs=2) as xpool, \
             tc_ctx.tile_pool(name="p1_rope", bufs=2) as rpool, \
             tc_ctx.tile_pool(name="p1_ps", bufs=4, space="PSUM") as pspool, \
             tc_ctx.tile_pool(name="p1_out", bufs=2) as opool, \
             tc_ctx.tile_pool(name="p1_sh", bufs=2) as shpool:
            wq_sb = wpool.tile([128, HL + 2, N_HT, 128], BF16)
            for ot in range(HL + 2):
                # Weights ride the scalar-engine HWDGE queue so the first X
                # chunk (sync queue) lands in parallel.
                nc.scalar.dma_start(out=wq_sb[:, ot], in_=wqkvt.ap()[:, ot])
            for ch in range(n_tc):
                xt_sb = xpool.tile([128, N_HT, TC], BF16)
                for hq in range(2):
                    nc.sync.dma_start(
                        out=xt_sb[:, hq * 16:(hq + 1) * 16, :],
                        in_=xt.ap()[:, ch, hq * 16:(hq + 1) * 16, :],
                    )
                rope_sb = rpool.tile([128, 4, TC], BF16)
                nc.sync.dma_start(
                    out=rope_sb,
                    in_=ropes.ap()[:, ch],
                )
                for ot in range(HL + 2):
                    ps = pspool.tile([128, TC], F32)
                    for h in range(N_HT):
                        nc.tensor.matmul(
                            ps,
                            lhsT=wq_sb[:, ot, h, :],
                            rhs=xt_sb[:, h, :],
                            start=(h == 0),
                            stop=(h == N_HT - 1),
                        )
                    if ot < HL + 1:
                        # RoPE for Q (ot<HL) and K (ot==HL).
                        cos_i = 0 if ot < HL else 2
                        sh = shpool.tile([128, TC], F32, tag="sh")
                        nc.vector.tensor_copy(sh[0:64, :], ps[64:128, :])
                        nc.vector.tensor_copy(sh[64:128, :], ps[0:64, :])
                        nc.vector.tensor_mul(sh, sh, rope_sb[:, cos_i + 1, :])
                        tmp = shpool.tile([128, TC], F32, tag="tmp")
                        nc.vector.tensor_mul(tmp, ps, rope_sb[:, cos_i, :])
                        qk_out = opool.tile([128, TC], BF16, tag="qk")
                        nc.vector.tensor_add(qk_out, tmp, sh)
                        nc.sync.dma_start(
                            out=qkt[ot * 128:(ot + 1) * 128,
                                    ch * TC:(ch + 1) * TC],
                            in_=qk_out,
                        )
                    else:
                        # V head: store [d, t] as-is; transposed on load later.
                        v_out = opool.tile([128, TC], BF16, tag="v")
                        nc.vector.tensor_copy(v_out, ps)
                        nc.sync.dma_start(
                            out=qkt[ot * 128:(ot + 1) * 128,
                                    ch * TC:(ch + 1) * TC],
                            in_=v_out,
                        )
                if ch == (S // TC) - 1:
                    # Batch-0 rows of qkt are complete: prefetch its K/V/Q now
                    # so attention can start the moment phase 1 drains.
                    load_kvq(0)

        # ------- Phases 2-3: attention + chunked AllToAll + out-proj --------
        with tc_ctx.tile_pool(name="p2_p", bufs=4) as ppool, \
             tc_ctx.tile_pool(name="p2_cs", bufs=2) as cspool, \
             tc_ctx.tile_pool(name="p2_ps_s", bufs=3, space="PSUM") as ps_s, \
             tc_ctx.tile_pool(name="p2_ps_o", bufs=2, space="PSUM") as ps_o, \
             tc_ctx.tile_pool(name="p2_ps_b", bufs=1, space="PSUM") as ps_b, \
             tc_ctx.tile_pool(name="p2_misc", bufs=2) as mpool, \
             tc_ctx.tile_pool(name="p3_w", bufs=3) as wspool, \
             tc_ctx.tile_pool(name="p3_a", bufs=1) as aopool, \
             tc_ctx.tile_pool(name="p3_st", bufs=1) as stpool, \
             tc_ctx.tile_pool(name="p3_res", bufs=3) as respool, \
             tc_ctx.tile_pool(name="p3_ps", bufs=2, space="PSUM") as pspool4:
            load_kvq(1)

            ao_sbs = {}
            st_sbs = {}

            def load_ao(b, half):
                ao = aopool.tile([128, 16, OWN], BF16, tag=f"ao{b}{half}",
                                 name=f"ao{b}{half}")
                nc.sync.dma_start(
                    out=ao,
                    in_=a2a_outs[b][half].rearrange(
                        "s (j p) t -> p (s j) t", p=128),
                )
                ao_sbs[(b, half)] = ao

            def emit_chain(b, half, jt):
                # Half an o_proj column-tile: contract over the 16 head-dim
                # tiles delivered by AllToAll (b, half). The lo half parks
                # its partial sums in SBUF; the hi half adds them back in.
                wos = wspool.tile([128, 16, 128], BF16, tag="wos")
                nc.scalar.dma_start(out=wos, in_=wot.ap()[:, jt, half])
                ps4 = pspool4.tile([128, OWN], F32)
                ao = ao_sbs[(b, half)]
                for h in range(16):
                    nc.tensor.matmul(
                        ps4,
                        lhsT=wos[:, h, :],
                        rhs=ao[:, h, :],
                        start=(h == 0),
                        stop=(h == 15),
                    )
                if half == 0:
                    if b not in st_sbs:
                        st_sbs[b] = stpool.tile([128, N_JT, OWN], BF16,
                                                tag="st", name=f"st{b}")
                    nc.vector.tensor_copy(st_sbs[b][:, jt, :], ps4)
                else:
                    res4 = respool.tile([128, OWN], F32, tag="res4")
                    nc.vector.tensor_add(res4, ps4, st_sbs[b][:, jt, :])
                    nc.sync.dma_start(
                        out=out_t[jt * 128:(jt + 1) * 128,
                                  b * OWN:(b + 1) * OWN],
                        in_=res4,
                    )

            class Grp:
                """One (batch, q-chunk, head) softmax group with its own
                score-matmul pipeline state."""

                def __init__(self, b, qc, hh):
                    self.b, self.qc, self.hh = b, qc, hh
                    self.kt_max = (QC // 128) * qc + (QC // 128)
                    self.pso = ps_o.tile([128, QC], F32, name="pso")
                    self.cs_e = cspool.tile([128, QC], BF16, tag="cse")
                    self.cs_o = cspool.tile([128, QC], BF16, tag="cso")
                    self.scores = []
                    self.next_kt = 0

                def emit_score(self):
                    # Diagonal blocks d_off>=1 only have live queries in
                    # columns [d_off*128:]; skip the dead columns.
                    kt = self.next_kt
                    self.next_kt += 1
                    k_sb, v_sb, q_sb = kvq[self.b]
                    d = kt - (QC // 128) * self.qc
                    off = d * 128 if (d >= 1 and kt >= 2) else 0
                    ps = ps_s.tile([128, QC], F32, name="ps")
                    nc.tensor.matmul(
                        ps[:, off:],
                        lhsT=k_sb[:, kt * 128:(kt + 1) * 128],
                        rhs=q_sb[:, self.hh,
                                 self.qc * QC + off:(self.qc + 1) * QC],
                        start=True,
                        stop=True,
                    )
                    self.scores.append((ps, off))

            def run_group(cur, nxt):
                """Consume cur's scores, pipelining: the next score matmul is
                issued to the in-order PE queue BEFORE the exp-dependent AV
                matmul so the PE never idles behind the scalar engine; near
                the group end the NEXT group's first scores are issued so the
                sums/recip boundary chain doesn't stall the PE either."""
                b, qc, hh = cur.b, cur.qc, cur.hh
                k_sb, v_sb, q_sb = kvq[b]
                for kt in range(cur.kt_max):
                    if cur.next_kt < cur.kt_max:
                        cur.emit_score()
                    elif nxt is not None and nxt.next_kt < 2:
                        nxt.emit_score()
                    ps, off = cur.scores.pop(0)
                    pt = ppool.tile([128, QC], BF16)
                    nc.scalar.activation(pt[:, off:], ps[:, off:], EXP)
                    d_off = kt - (QC // 128) * qc
                    if d_off >= 0:
                        nc.vector.tensor_mul(
                            pt[:, off:], pt[:, off:], masks[:, d_off, off:]
                        )
                    nc.tensor.matmul(
                        cur.pso[:, off:],
                        lhsT=v_sb[:, kt, :],
                        rhs=pt[:, off:],
                        start=(kt == 0),
                        stop=(kt == cur.kt_max - 1),
                        skip_group_check=True,
                    )
                    # Softmax denominator: two bf16 partial sums (even/odd
                    # kt) keep the DVE in 2x 16-bit mode and halve the
                    # serial dependence chain.
                    cs = cur.cs_e if kt % 2 == 0 else cur.cs_o
                    if kt < 2:
                        nc.vector.tensor_copy(cs, pt)
                    else:
                        nc.vector.tensor_add(
                            cs[:, off:], cs[:, off:], pt[:, off:]
                        )
                if nxt is not None and nxt.next_kt < 2:
                    nxt.emit_score()
                # Sum the two partials over partitions and broadcast to all
                # 128 partitions with one accumulating matmul pair against
                # the all-ones stationary.
                sums_bc = ps_b.tile([128, QC], F32)
                nc.tensor.matmul(
                    sums_bc, lhsT=ones_mat, rhs=cur.cs_e,
                    start=True, stop=False,
                )
                nc.tensor.matmul(
                    sums_bc, lhsT=ones_mat, rhs=cur.cs_o,
                    start=False, stop=True,
                )
                recip_bc = mpool.tile([128, QC], F32, tag="recip_bc")
                rscr = mpool.tile([128, QC], F32, tag="rscr")
                nc.vector.reciprocal_approx_accurate(recip_bc, sums_bc, rscr)
                attn = mpool.tile([128, QC], BF16, tag="attn")
                nc.vector.tensor_mul(attn, cur.pso, recip_bc)
                # Scatter the two 256-token halves to their owners' AllToAll
                # source blocks (per head-pair half).
                for sub in range(2):
                    nc.sync.dma_start(
                        out=a2a_ins[b][hh // 2][
                            2 * qc + sub,
                            (hh % 2) * 128:(hh % 2 + 1) * 128, :],
                        in_=attn[:, sub * OWN:(sub + 1) * OWN],
                    )

            # Head-pair-chunked re-shard: heads are iterated hh-major, so
            # heads {0,1} finish at batch midpoint and their AllToAll (and
            # the o_proj chains it feeds) overlaps the {2,3} attention.
            lagged = []      # (activation_group, b, half) fired A2As
            avail = []       # chains ready to issue, FIFO
            CHAIN_LAG = 3    # groups between A2A fire and first chain issue
            group_no = 0
            for b in range(B):
                coords = [(b, qc, hh) for hh in range(HL)
                          for qc in reversed(range(n_qc))]
                cur = Grp(*coords[0])
                cur.emit_score()
                cur.emit_score()
                for gi in range(len(coords)):
                    nxt = Grp(*coords[gi + 1]) if gi + 1 < len(coords) else None
                    run_group(cur, nxt)
                    group_no += 1
                    if gi == 7 or gi == 15:
                        half = 0 if gi == 7 else 1
                        nc.gpsimd.collective_compute(
                            "AllToAll",
                            mybir.AluOpType.bypass,
                            replica_groups=[list(range(N_CORES))],
                            ins=[a2a_ins[b][half][:]],
                            outs=[a2a_outs[b][half][:]],
                        )
                        load_ao(b, half)
                        lagged.append((group_no + CHAIN_LAG, b, half))
                    while lagged and lagged[0][0] <= group_no:
                        _, bb, hf = lagged.pop(0)
                        avail.extend((bb, hf, jt) for jt in range(N_JT))
                    # Interleave ready o_proj half-chains into attention so
                    # the PE stays fed while scalar/vector run softmax.
                    for _ in range(6):
                        if avail:
                            emit_chain(*avail.pop(0))
                    cur = nxt
            for _, bb, hf in lagged:
                avail.extend((bb, hf, jt) for jt in range(N_JT))
            for ch in avail:
                emit_chain(*ch)


def _build_program():
    nc = bacc.Bacc("TRN2", target_bir_lowering=False, debug=False,
                   num_devices=N_CORES)
    xt = nc.declare_dram_parameter("xt", [128, T // TC, N_HT, TC], BF16,
                                   isOutput=False)
    wqkvt = nc.declare_dram_parameter("wqkvt", [128, HL + 2, N_HT, 128], BF16,
                                      isOutput=False)
    wot = nc.declare_dram_parameter("wot", [128, N_JT, 2, 16, 128], BF16,
                                    isOutput=False)
    ropes = nc.declare_dram_parameter("ropes", [128, T // TC, 4, TC], BF16,
                                      isOutput=False)
    out_t = nc.declare_dram_parameter("out_t", [HID, B * OWN], F32,
                                      isOutput=True)

    qkt = nc.dram_tensor("qkt", [QKV_ROWS, T], BF16).ap()
    a2a_ins = [[nc.dram_tensor(f"a2a_in{b}_{h}", [N_CORES, 2 * D, OWN],
                               BF16).ap()
                for h in range(2)] for b in range(B)]
    a2a_outs = [[nc.dram_tensor(f"a2a_out{b}_{h}", [N_CORES, 2 * D, OWN],
                                BF16).ap()
                 for h in range(2)] for b in range(B)]

    with tile.TileContext(nc) as tc_ctx:
        _emit(tc_ctx, xt, wqkvt, wot, ropes, out_t, qkt, a2a_ins, a2a_outs)
    nc.finalize()
    return nc


def _host_inputs(hidden_states, w_qkv, w_o):
    """Shard + transpose inputs for the 8 cores; returns in_maps."""
    X = np.asarray(hidden_states, dtype=np.float32).reshape(T, HID)
    # [p, ch, ht, tc] tiled layout so every DMA line is contiguous.
    xt = np.ascontiguousarray(
        X.reshape(T // TC, TC, N_HT, 128).transpose(3, 0, 2, 1)
    ).astype(ml_dtypes.bfloat16)

    # RoPE tables in [d, t] layout with rotate-half sign folded into sin and
    # the attention scale folded into the Q tables.
    inv_freq = 1.0 / (ROPE_BASE ** (np.arange(0, D, 2, dtype=np.float32) / D))
    pos = np.arange(S, dtype=np.float32)
    freqs = np.outer(pos, inv_freq)                      # (S, D/2)
    emb = np.concatenate([freqs, freqs], axis=-1)        # (S, D)
    cos = np.cos(emb).T.astype(np.float32)               # (D, S)
    sin = np.sin(emb).T.astype(np.float32)
    sgn = np.concatenate([-np.ones(D // 2), np.ones(D // 2)]).astype(np.float32)
    sins = sgn[:, None] * sin
    cos_t = np.tile(cos, (1, B))                         # (D, T)
    sins_t = np.tile(sins, (1, B))
    scale = np.float32(D ** -0.5)
    ropes = np.stack([cos_t * scale, sins_t * scale, cos_t, sins_t], axis=0)
    ropes = np.ascontiguousarray(
        ropes.reshape(4, 128, T // TC, TC).transpose(1, 2, 0, 3),
    ).astype(ml_dtypes.bfloat16)

    w_qkv = np.asarray(w_qkv, dtype=np.float32)
    w_o = np.asarray(w_o, dtype=np.float32)
    # Full w_o, tiled [p, jt, half, s*2+j, c] = w_o[jt*128+c, g*128+p] with
    # g = 4*s + 2*half + j: every core computes all output columns for its
    # own tokens, contracting the AllToAll half-blocks in arrival order.
    wot = np.ascontiguousarray(
        w_o.reshape(N_JT, 128, N_CORES, 2, 2, 128).transpose(5, 0, 3, 2, 4, 1)
    ).reshape(128, N_JT, 2, 16, 128).astype(ml_dtypes.bfloat16)
    q_sz = N_HEADS * D
    kv_sz = N_KV_HEADS * D
    in_maps = []
    for c in range(N_CORES):
        qr = w_qkv[c * HL * D:(c + 1) * HL * D]
        kr = w_qkv[q_sz + c * D:q_sz + (c + 1) * D]
        vr = w_qkv[q_sz + kv_sz + c * D:q_sz + kv_sz + (c + 1) * D]
        w_shard = np.concatenate([qr, kr, vr], axis=0)           # (768, HID)
        wqkvt_c = np.ascontiguousarray(
            w_shard.reshape(HL + 2, 128, N_HT, 128).transpose(3, 0, 2, 1)
        ).astype(ml_dtypes.bfloat16)
        in_maps.append({
            "xt": xt, "wqkvt": wqkvt_c, "wot": wot, "ropes": ropes,
        })
    return in_maps


def _run(hidden_states, w_qkv, w_o, trace=False, tmpdir=None):
    in_maps = _host_inputs(hidden_states, w_qkv, w_o)
    nc = _build_program()
    res = run_bass_kernel_spmd(nc, in_maps, list(range(N_CORES)),
                               trace=trace, tmpdir=tmpdir)
    out = np.empty((B, S, HID), dtype=np.float32)
    for c in range(N_CORES):
        o = np.asarray(res.results[c]["out_t"])          # (HID, B*OWN)
        for b in range(B):
            out[b, c * OWN:(c + 1) * OWN, :] = o[:, b * OWN:(b + 1) * OWN].T
    return out, res


def kernel(hidden_states, w_qkv, w_o):
    out, _ = _run(hidden_states, w_qkv, w_o, trace=False)
    return out


# revision 19
# speedup vs baseline: 1.0095x; 1.0095x over previous
"""Trainium2 Bass kernel: dense transformer attention block (QKV proj + RoPE +
GQA causal attention + output proj), tensor-parallel over 8 NeuronCores.

Sharding: heads are split across cores (4 Q heads + 1 KV head per core).
Each core computes its QKV shard for all tokens and runs attention for its
heads. The per-core attention outputs (head-sharded) are then re-sharded to
token-sharded via one AllToAll per batch (each core keeps a 256-token slice
of every batch), after which each core computes the FULL output projection
(all 4096 columns) for its own tokens, streaming w_o from HBM. The host
concatenates the per-core token slices — no AllGather / AllReduce needed,
cutting collective traffic 8x vs gathering the full attention output.

All token-indexed tensors live on-device in transposed layout
([feature, token]) so the hidden-dim contraction lands on the partition axis
for the TensorEngine; the host transposes inputs/outputs during
shard/unshard.
"""

from contextlib import ExitStack

import numpy as np
import ml_dtypes

import concourse.bass as bass
from concourse import bacc
import concourse.tile as tile
import concourse.mybir as mybir
from concourse.bass_utils import run_bass_kernel_spmd

F32 = mybir.dt.float32
F32R = mybir.dt.float32r
BF16 = mybir.dt.bfloat16
EXP = mybir.ActivationFunctionType.Exp

N_CORES = 8
N_HEADS = 32
N_KV_HEADS = 8
D = 128          # head dim
HID = 4096
B = 2
S = 2048
T = B * S        # 4096 tokens
ROPE_BASE = 10000.0

HL = N_HEADS // N_CORES          # 4 local Q heads per core
QKV_ROWS = (HL + 2) * D          # 768: 4 Q heads + 1 K head + 1 V head
OWN = S // N_CORES               # 256 tokens owned per core per batch

TC = 512                         # token chunk for the QKV projection phase
QC = 512                         # query chunk in attention
N_HT = HID // 128                # 32 hidden tiles
N_JT = HID // 128                # 32 output column tiles in o_proj


def _emit(tc_ctx, xt, wqkvt, wot, ropes, out_t, qkt, a2a_ins, a2a_outs):
    nc = tc_ctx.nc
    n_tc = T // TC           # 8 projection chunks
    n_kt = S // 128          # 16 k-tiles per batch
    n_qc = S // QC           # 4 q-chunks per batch

    with ExitStack() as es:
        const_pool = es.enter_context(tc_ctx.tile_pool(name="const", bufs=1))
        # All-ones stationary: one matmul sums colsum partials over partitions
        # AND broadcasts the result across all 128 partitions.
        ones_mat = const_pool.tile([128, 128], BF16)
        # Diagonal causal masks: mask_d[k, q] = 1.0 if q - k - 128*d >= 0.
        masks = const_pool.tile([128, 4, QC], BF16)
        nc.vector.memset(masks, 1.0)
        nc.vector.tensor_copy(ones_mat, masks[:, 0, 0:128])
        for d_off in range(4):
            nc.gpsimd.affine_select(
                out=masks[:, d_off, :],
                in_=masks[:, d_off, :],
                compare_op=mybir.AluOpType.is_ge,
                fill=0.0,
                base=-128 * d_off,
                pattern=[[1, QC]],
                channel_multiplier=-1,
            )

        # Pools for attention inputs, opened early so batch-0 K/V/Q loads can
        # overlap the tail of phase 1.
        qpool = es.enter_context(tc_ctx.tile_pool(name="p2_q", bufs=2))
        kvpool = es.enter_context(tc_ctx.tile_pool(name="p2_kv", bufs=2))
        kvq = {}

        def load_kvq(b):
            k_sb = kvpool.tile([128, S], BF16, tag="k", name=f"k_sb{b}")
            nc.sync.dma_start(
                out=k_sb, in_=qkt[HL * 128:(HL + 1) * 128, b * S:(b + 1) * S]
            )
            v_sb = kvpool.tile([128, n_kt, 128], BF16, tag="v", name=f"v_sb{b}")
            for kt in range(n_kt):
                nc.scalar.dma_start(
                    out=v_sb[:, kt, :],
                    in_=qkt[(HL + 1) * 128:(HL + 2) * 128,
                            b * S + kt * 128:b * S + (kt + 1) * 128],
                    transpose=True,
                )
            q_sb = qpool.tile([128, HL, S], BF16, tag="q", name=f"q_sb{b}")
            nc.sync.dma_start(
                out=q_sb,
                in_=qkt.rearrange("(hh p) t -> p hh t", p=128)[
                    :, 0:HL, b * S:(b + 1) * S
                ],
            )
            kvq[b] = (k_sb, v_sb, q_sb)

        # ---------------- Phase 1: QKV projection + RoPE --------------------
        with tc_ctx.tile_pool(name="p1_w", bufs=1) as wpool, \
             tc_ctx.tile_pool(name="p1_x", bufs=2) as xpool, \
             tc_ctx.tile_pool(name="p1_rope", bufs=2) as rpool, \
             tc_ctx.tile_pool(name="p1_ps", bufs=4, space="PSUM") as pspool, \
             tc_ctx.tile_pool(name="p1_out", bufs=2) as opool, \
             tc_ctx.tile_pool(name="p1_sh", bufs=2) as shpool:
            wq_sb = wpool.tile([128, HL + 2, N_HT, 128], BF16)
            for ot in range(HL + 2):
                # Weights ride the scalar-engine HWDGE queue so the first X
                # chunk (sync queue) lands in parallel.
                nc.scalar.dma_start(out=wq_sb[:, ot], in_=wqkvt.ap()[:, ot])
            for ch in range(n_tc):
                xt_sb = xpool.tile([128, N_HT, TC], BF16)
                for hq in range(2):
                    nc.sync.dma_start(
                        out=xt_sb[:, hq * 16:(hq + 1) * 16, :],
                        in_=xt.ap()[:, ch, hq * 16:(hq + 1) * 16, :],
                    )
                rope_sb = rpool.tile([128, 4, TC], BF16)
                nc.sync.dma_start(
                    out=rope_sb,
                    in_=ropes.ap()[:, ch],
                )
                for ot in range(HL + 2):
                    ps = pspool.tile([128, TC], F32)
                    for h in range(N_HT):
                        nc.tensor.matmul(
                            ps,
                            lhsT=wq_sb[:, ot, h, :],
                            rhs=xt_sb[:, h, :],
                            start=(h == 0),
                            stop=(h == N_HT - 1),
                        )
                    if ot < HL + 1:
                        # RoPE for Q (ot<HL) and K (ot==HL).
                        cos_i = 0 if ot < HL else 2
                        sh = shpool.tile([128, TC], F32, tag="sh")
                        nc.vector.tensor_copy(sh[0:64, :], ps[64:128, :])
                        nc.vector.tensor_copy(sh[64:128, :], ps[0:64, :])
                        nc.vector.tensor_mul(sh, sh, rope_sb[:, cos_i + 1, :])
                        tmp = shpool.tile([128, TC], F32, tag="tmp")
                        nc.vector.tensor_mul(tmp, ps, rope_sb[:, cos_i, :])
                        qk_out = opool.tile([128, TC], BF16, tag="qk")
                        nc.vector.tensor_add(qk_out, tmp, sh)
                        nc.sync.dma_start(
                            out=qkt[ot * 128:(ot + 1) * 128,
                                    ch * TC:(ch + 1) * TC],
                            in_=qk_out,
                        )
                    else:
                        # V head: store [d, t] as-is; transposed on load later.
                        v_out = opool.tile([128, TC], BF16, tag="v")
                        nc.vector.tensor_copy(v_out, ps)
                        nc.sync.dma_start(
                            out=qkt[ot * 128:(ot + 1) * 128,
                                    ch * TC:(ch + 1) * TC],
                            in_=v_out,
                        )
                if ch == (S // TC) - 1:
                    # Batch-0 rows of qkt are complete: prefetch its K/V/Q now
                    # so attention can start the moment phase 1 drains.
                    load_kvq(0)

        # ------- Phases 2-3: attention + chunked AllToAll + out-proj --------
        with tc_ctx.tile_pool(name="p2_p", bufs=4) as ppool, \
             tc_ctx.tile_pool(name="p2_cs", bufs=2) as cspool, \
             tc_ctx.tile_pool(name="p2_ps_s", bufs=3, space="PSUM") as ps_s, \
             tc_ctx.tile_pool(name="p2_ps_o", bufs=2, space="PSUM") as ps_o, \
             tc_ctx.tile_pool(name="p2_ps_b", bufs=1, space="PSUM") as ps_b, \
             tc_ctx.tile_pool(name="p2_misc", bufs=2) as mpool, \
             tc_ctx.tile_pool(name="p3_w", bufs=3) as wspool, \
             tc_ctx.tile_pool(name="p3_a", bufs=1) as aopool, \
             tc_ctx.tile_pool(name="p3_st", bufs=1) as stpool, \
             tc_ctx.tile_pool(name="p3_res", bufs=3) as respool, \
             tc_ctx.tile_pool(name="p3_ps", bufs=2, space="PSUM") as pspool4:
            load_kvq(1)

            ao_sbs = {}
            st_sbs = {}

            def load_ao(b, half):
                ao = aopool.tile([128, 16, OWN], BF16, tag=f"ao{b}{half}",
                                 name=f"ao{b}{half}")
                nc.sync.dma_start(
                    out=ao,
                    in_=a2a_outs[b][half].rearrange(
                        "s (j p) t -> p (s j) t", p=128),
                )
                ao_sbs[(b, half)] = ao

            def emit_chain(b, half, jt):
                # Half an o_proj column-tile: contract over the 16 head-dim
                # tiles delivered by AllToAll (b, half). The lo half parks
                # its partial sums in SBUF; the hi half adds them back in.
                wos = wspool.tile([128, 16, 128], BF16, tag="wos")
                nc.scalar.dma_start(out=wos, in_=wot.ap()[:, jt, half])
                ps4 = pspool4.tile([128, OWN], F32)
                ao = ao_sbs[(b, half)]
                for h in range(16):
                    nc.tensor.matmul(
                        ps4,
                        lhsT=wos[:, h, :],
                        rhs=ao[:, h, :],
                        start=(h == 0),
                        stop=(h == 15),
                    )
                if half == 0:
                    if b not in st_sbs:
                        st_sbs[b] = stpool.tile([128, N_JT, OWN], BF16,
                                                tag="st", name=f"st{b}")
                    nc.vector.tensor_copy(st_sbs[b][:, jt, :], ps4)
                else:
                    res4 = respool.tile([128, OWN], F32, tag="res4")
                    nc.vector.tensor_add(res4, ps4, st_sbs[b][:, jt, :])
                    nc.sync.dma_start(
                        out=out_t[jt * 128:(jt + 1) * 128,
                                  b * OWN:(b + 1) * OWN],
                        in_=res4,
                    )

            class Grp:
                """One (batch, q-chunk, head) softmax group with its own
                score-matmul pipeline state."""

                def __init__(self, b, qc, hh):
                    self.b, self.qc, self.hh = b, qc, hh
                    self.kt_max = (QC // 128) * qc + (QC // 128)
                    self.pso = ps_o.tile([128, QC], F32, name="pso")
                    self.cs_e = cspool.tile([128, QC], BF16, tag="cse")
                    self.cs_o = cspool.tile([128, QC], BF16, tag="cso")
                    self.scores = []
                    self.next_kt = 0

                def emit_score(self):
                    # Diagonal blocks d_off>=1 only have live queries in
                    # columns [d_off*128:]; skip the dead columns.
                    kt = self.next_kt
                    self.next_kt += 1
                    k_sb, v_sb, q_sb = kvq[self.b]
                    d = kt - (QC // 128) * self.qc
                    off = d * 128 if (d >= 1 and kt >= 2) else 0
                    ps = ps_s.tile([128, QC], F32, name="ps")
                    nc.tensor.matmul(
                        ps[:, off:],
                        lhsT=k_sb[:, kt * 128:(kt + 1) * 128],
                        rhs=q_sb[:, self.hh,
                                 self.qc * QC + off:(self.qc + 1) * QC],
                        start=True,
                        stop=True,
                    )
                    self.scores.append((ps, off))

            def run_group(cur, nxt):
                """Consume cur's scores, pipelining: the next score matmul is
                issued to the in-order PE queue BEFORE the exp-dependent AV
                matmul so the PE never idles behind the scalar engine; near
                the group end the NEXT group's first scores are issued so the
                sums/recip boundary chain doesn't stall the PE either."""
                b, qc, hh = cur.b, cur.qc, cur.hh
                k_sb, v_sb, q_sb = kvq[b]
                for kt in range(cur.kt_max):
                    if cur.next_kt < cur.kt_max:
                        cur.emit_score()
                    elif nxt is not None and nxt.next_kt < 2:
                        nxt.emit_score()
                    ps, off = cur.scores.pop(0)
                    pt = ppool.tile([128, QC], BF16)
                    nc.scalar.activation(pt[:, off:], ps[:, off:], EXP)
                    d_off = kt - (QC // 128) * qc
                    if d_off >= 0:
                        nc.vector.tensor_mul(
                            pt[:, off:], pt[:, off:], masks[:, d_off, off:]
                        )
                    nc.tensor.matmul(
                        cur.pso[:, off:],
                        lhsT=v_sb[:, kt, :],
                        rhs=pt[:, off:],
                        start=(kt == 0),
                        stop=(kt == cur.kt_max - 1),
                        skip_group_check=True,
                    )
                    # Softmax denominator: two bf16 partial sums (even/odd
                    # kt) keep the DVE in 2x 16-bit mode and halve the
                    # serial dependence chain.
                    cs = cur.cs_e if kt % 2 == 0 else cur.cs_o
                    if kt < 2:
                        nc.vector.tensor_copy(cs, pt)
                    else:
                        nc.vector.tensor_add(
                            cs[:, off:], cs[:, off:], pt[:, off:]
                        )
                if nxt is not None and nxt.next_kt < 2:
                    nxt.emit_score()
                # Sum the two partials over partitions and broadcast to all
                # 128 partitions with one accumulating matmul pair against
                # the all-ones stationary.
                sums_bc = ps_b.tile([128, QC], F32)
                nc.tensor.matmul(
                    sums_bc, lhsT=ones_mat, rhs=cur.cs_e,
                    start=True, stop=False,
                )
                nc.tensor.matmul(
                    sums_bc, lhsT=ones_mat, rhs=cur.cs_o,
                    start=False, stop=True,
                )
                recip_bc = mpool.tile([128, QC], F32, tag="recip_bc")
                rscr = mpool.tile([128, QC], F32, tag="rscr")
                nc.vector.reciprocal_approx_accurate(recip_bc, sums_bc, rscr)
                attn = mpool.tile([128, QC], BF16, tag="attn")
                nc.vector.tensor_mul(attn, cur.pso, recip_bc)
                # Scatter the two 256-token halves to their owners' AllToAll
                # source blocks (per head-pair half).
                for sub in range(2):
                    nc.sync.dma_start(
                        out=a2a_ins[b][hh // 2][
                            2 * qc + sub,
                            (hh % 2) * 128:(hh % 2 + 1) * 128, :],
                        in_=attn[:, sub * OWN:(sub + 1) * OWN],
                    )

            # Head-pair-chunked re-shard: heads are iterated hh-major, so
            # heads {0,1} finish at batch midpoint and their AllToAll (and
            # the o_proj chains it feeds) overlaps the {2,3} attention.
            lagged = []      # (activation_group, b, half) fired A2As
            avail = []       # chains ready to issue, FIFO
            CHAIN_LAG = 5    # groups between A2A fire and first chain issue
            group_no = 0
            for b in range(B):
                coords = [(b, qc, hh) for hh in range(HL)
                          for qc in reversed(range(n_qc))]
                cur = Grp(*coords[0])
                cur.emit_score()
                cur.emit_score()
                for gi in range(len(coords)):
                    nxt = Grp(*coords[gi + 1]) if gi + 1 < len(coords) else None
                    run_group(cur, nxt)
                    group_no += 1
                    if gi == 7 or gi == 15:
                        half = 0 if gi == 7 else 1
                        nc.gpsimd.collective_compute(
                            "AllToAll",
                            mybir.AluOpType.bypass,
                            replica_groups=[list(range(N_CORES))],
                            ins=[a2a_ins[b][half][:]],
                            outs=[a2a_outs[b][half][:]],
                        )
                        load_ao(b, half)
                        lagged.append((group_no + CHAIN_LAG, b, half))
                    while lagged and lagged[0][0] <= group_no:
                        _, bb, hf = lagged.pop(0)
                        avail.extend((bb, hf, jt) for jt in range(N_JT))
                    # Interleave ready o_proj half-chains into attention so
                    # the PE stays fed while scalar/vector run softmax. Near
                    # the end, throttle so enough chains remain to cover the
                    # final AllToAll's latency.
                    for _ in range(3 if group_no >= 26 else 6):
                        if avail:
                            emit_chain(*avail.pop(0))
                    cur = nxt
            for _, bb, hf in lagged:
                avail.extend((bb, hf, jt) for jt in range(N_JT))
            for ch in avail:
                emit_chain(*ch)


def _build_program():
    nc = bacc.Bacc("TRN2", target_bir_lowering=False, debug=False,
                   num_devices=N_CORES)
    xt = nc.declare_dram_parameter("xt", [128, T // TC, N_HT, TC], BF16,
                                   isOutput=False)
    wqkvt = nc.declare_dram_parameter("wqkvt", [128, HL + 2, N_HT, 128], BF16,
                                      isOutput=False)
    wot = nc.declare_dram_parameter("wot", [128, N_JT, 2, 16, 128], BF16,
                                    isOutput=False)
    ropes = nc.declare_dram_parameter("ropes", [128, T // TC, 4, TC], BF16,
                                      isOutput=False)
    out_t = nc.declare_dram_parameter("out_t", [HID, B * OWN], F32,
                                      isOutput=True)

    qkt = nc.dram_tensor("qkt", [QKV_ROWS, T], BF16).ap()
    a2a_ins = [[nc.dram_tensor(f"a2a_in{b}_{h}", [N_CORES, 2 * D, OWN],
                               BF16).ap()
                for h in range(2)] for b in range(B)]
    a2a_outs = [[nc.dram_tensor(f"a2a_out{b}_{h}", [N_CORES, 2 * D, OWN],
                                BF16).ap()
                 for h in range(2)] for b in range(B)]

    with tile.TileContext(nc) as tc_ctx:
        _emit(tc_ctx, xt, wqkvt, wot, ropes, out_t, qkt, a2a_ins, a2a_outs)
    nc.finalize()
    return nc


def _host_inputs(hidden_states, w_qkv, w_o):
    """Shard + transpose inputs for the 8 cores; returns in_maps."""
    X = np.asarray(hidden_states, dtype=np.float32).reshape(T, HID)
    # [p, ch, ht, tc] tiled layout so every DMA line is contiguous.
    xt = np.ascontiguousarray(
        X.reshape(T // TC, TC, N_HT, 128).transpose(3, 0, 2, 1)
    ).astype(ml_dtypes.bfloat16)

    # RoPE tables in [d, t] layout with rotate-half sign folded into sin and
    # the attention scale folded into the Q tables.
    inv_freq = 1.0 / (ROPE_BASE ** (np.arange(0, D, 2, dtype=np.float32) / D))
    pos = np.arange(S, dtype=np.float32)
    freqs = np.outer(pos, inv_freq)                      # (S, D/2)
    emb = np.concatenate([freqs, freqs], axis=-1)        # (S, D)
    cos = np.cos(emb).T.astype(np.float32)               # (D, S)
    sin = np.sin(emb).T.astype(np.float32)
    sgn = np.concatenate([-np.ones(D // 2), np.ones(D // 2)]).astype(np.float32)
    sins = sgn[:, None] * sin
    cos_t = np.tile(cos, (1, B))                         # (D, T)
    sins_t = np.tile(sins, (1, B))
    scale = np.float32(D ** -0.5)
    ropes = np.stack([cos_t * scale, sins_t * scale, cos_t, sins_t], axis=0)
    ropes = np.ascontiguousarray(
        ropes.reshape(4, 128, T // TC, TC).transpose(1, 2, 0, 3),
    ).astype(ml_dtypes.bfloat16)

    w_qkv = np.asarray(w_qkv, dtype=np.float32)
    w_o = np.asarray(w_o, dtype=np.float32)
    # Full w_o, tiled [p, jt, half, s*2+j, c] = w_o[jt*128+c, g*128+p] with
    # g = 4*s + 2*half + j: every core computes all output columns for its
    # own tokens, contracting the AllToAll half-blocks in arrival order.
    wot = np.ascontiguousarray(
        w_o.reshape(N_JT, 128, N_CORES, 2, 2, 128).transpose(5, 0, 3, 2, 4, 1)
    ).reshape(128, N_JT, 2, 16, 128).astype(ml_dtypes.bfloat16)
    q_sz = N_HEADS * D
    kv_sz = N_KV_HEADS * D
    in_maps = []
    for c in range(N_CORES):
        qr = w_qkv[c * HL * D:(c + 1) * HL * D]
        kr = w_qkv[q_sz + c * D:q_sz + (c + 1) * D]
        vr = w_qkv[q_sz + kv_sz + c * D:q_sz + kv_sz + (c + 1) * D]
        w_shard = np.concatenate([qr, kr, vr], axis=0)           # (768, HID)
        wqkvt_c = np.ascontiguousarray(
            w_shard.reshape(HL + 2, 128, N_HT, 128).transpose(3, 0, 2, 1)
        ).astype(ml_dtypes.bfloat16)
        in_maps.append({
            "xt": xt, "wqkvt": wqkvt_c, "wot": wot, "ropes": ropes,
        })
    return in_maps


def _run(hidden_states, w_qkv, w_o, trace=False, tmpdir=None):
    in_maps = _host_inputs(hidden_states, w_qkv, w_o)
    nc = _build_program()
    res = run_bass_kernel_spmd(nc, in_maps, list(range(N_CORES)),
                               trace=trace, tmpdir=tmpdir)
    out = np.empty((B, S, HID), dtype=np.float32)
    for c in range(N_CORES):
        o = np.asarray(res.results[c]["out_t"])          # (HID, B*OWN)
        for b in range(B):
            out[b, c * OWN:(c + 1) * OWN, :] = o[:, b * OWN:(b + 1) * OWN].T
    return out, res


def kernel(hidden_states, w_qkv, w_o):
    out, _ = _run(hidden_states, w_qkv, w_o, trace=False)
    return out
